# revision 2
# baseline (speedup 1.0000x reference)
"""Trainium2 Bass kernel for CantorGlobalAttention (sparse attention).

Math (per direction x, expert e, batch b):
  scores[p, k] = Q[x,e,b,p] * kappa[k]          (rank-1 outer product)
  kappa[k]     = K_aff[x, route(e,w), b, q] * fac(e,w) / temp,  k=(w,q)
  attn         = softmax_k(scores)
  out[p, :]    = attn @ V_neighbors[k, :]
  final        = sum_x softmax(fusion_weights)[x] * out_x

Device strategy (8 cores, expert-parallel, 2 experts/core, 40 (x,e,b)
tuples/core):
  - scores via PE outer products: stationary = kappa_T [6,128] (6 chunks of
    128 keys), moving = block-diagonalized Q [6, 1536] -> scores [128k, 1536]
    in PSUM (k on partitions = matmul-ready orientation).
  - one big ScalarE Exp per tuple: PSUM [128,1536] -> SBUF.
  - attn @ V as 12 accumulating matmuls (lhsT = exp chunk [128,128],
    rhs = V chunk [128,129] with a ones column appended so the softmax
    denominator Z falls out of the same matmul).
  - VectorE: reciprocal(Z), scale by wts[x]/Z, accumulate over x in SBUF.
  - host does all layout: neighbor gather, beta/temp folding into K,
    block-diag Q, fusion-weight softmax (tiny tensors only).
"""

import numpy as np

import concourse.bass as bass
import concourse.tile as tile
from concourse import bacc, mybir
from concourse.bass_utils import run_bass_kernel_spmd

F32 = mybir.dt.float32

NDIR = 5
E = 16
W = 3
D = 128
P = 256
B = 4
DEPTH = 8

N_CORES = 8
ELOC = E // N_CORES          # experts per core = 2
NT = NDIR * ELOC * B         # tuples per core = 40
NCH = W * 2                  # key chunks per tuple (w, half) = 6
KTOT = NCH * 128             # keys per tuple = 768
FREE_V = NCH * (D + 1)       # V stage free size = 774
NBLK = NT // 4               # tuple column blocks = 10

# dtype for the attn @ V matmul operands (exp tile + V). float32 is exact;
# bfloat16 halves weight-load time (enables FWL) at ~1e-3 rel error.
MM_DT = F32
MM_NP = np.float32


def _routes() -> np.ndarray:
    def cantor(pos: int) -> float:
        x = pos / max(1, E - 1)
        x = max(1e-06, min(x, 1.0 - 1e-06))
        val, factor = 0.0, 0.5
        for _ in range(DEPTH):
            x *= 3.0
            digit = int(x)
            x -= digit
            if digit == 2:
                val += factor
            factor *= 0.5
        return val

    coords = np.array([cantor(i) for i in range(E)], dtype=np.float32)
    routes = np.zeros((E, W), dtype=np.int32)
    for i in range(E):
        d = np.abs(coords - coords[i])
        routes[i] = np.sort(np.argsort(d, kind="stable")[:W])
    return routes


ROUTES = _routes()


def _tuple_iter():
    """(t, x, e_local, b) in x-major order (x outermost for fusion accum)."""
    t = 0
    for x in range(NDIR):
        for e in range(ELOC):
            for b in range(B):
                yield t, x, e, b
                t += 1


def _build_program():
    nc = bacc.Bacc(None)

    vd = nc.dram_tensor("v", [NT, 128, FREE_V], MM_DT, kind="ExternalInput")
    kd = nc.dram_tensor("k", [4, NCH, NBLK * 128], F32, kind="ExternalInput")
    qd = nc.dram_tensor("q", [4, NCH, NBLK * 1536], F32, kind="ExternalInput")
    wd = nc.dram_tensor("w", [128, NDIR], F32, kind="ExternalInput")
    od = nc.dram_tensor("o", [ELOC, B, 2, 128, 128], F32, kind="ExternalOutput")

    with tile.TileContext(nc) as tc:
        with (
            tc.tile_pool(name="const", bufs=1) as const,
            tc.tile_pool(name="vstream", bufs=4) as vpool,
            tc.tile_pool(name="exp", bufs=2) as epool,
            tc.tile_pool(name="small", bufs=4) as spool_small,
            tc.tile_pool(name="psum_s", bufs=2, space="PSUM") as pscore,
            tc.tile_pool(name="psum_o", bufs=2, space="PSUM") as pout,
        ):
            k_tile = const.tile([128, NBLK * 128], F32)
            q_tile = const.tile([128, NBLK * 1536], F32)
            wts_tile = const.tile([128, NDIR], F32)
            acc = const.tile([128, ELOC * B * 2 * 128], F32)

            nc.default_dma_engine.dma_start(wts_tile[:], wd[:])
            for g in range(4):
                nc.default_dma_engine.dma_start(
                    k_tile[32 * g : 32 * g + NCH, :], kd[g]
                )
                nc.default_dma_engine.dma_start(
                    q_tile[32 * g : 32 * g + NCH, :], qd[g]
                )

            for t, x, e, b in _tuple_iter():
                g, blk = t % 4, t // 4
                bp = 32 * g

                v = vpool.tile([128, FREE_V], MM_DT)
                nc.default_dma_engine.dma_start(v[:], vd[t])

                # scores [128k, 1536] = kappa_T.T @ Qdiag
                S = pscore.tile([128, 1536], F32)
                for m in range(3):
                    nc.tensor.matmul(
                        S[:, m * 512 : (m + 1) * 512],
                        k_tile[bp : bp + NCH, blk * 128 : (blk + 1) * 128],
                        q_tile[
                            bp : bp + NCH,
                            blk * 1536 + m * 512 : blk * 1536 + (m + 1) * 512,
                        ],
                        start=True,
                        stop=True,
                        tile_position=(bp, 0),
                    )

                Ex = epool.tile([128, 1536], MM_DT)
                nc.scalar.activation(Ex[:], S[:], mybir.ActivationFunctionType.Exp)

                # attended [p, d] (+ Z in col 128) accumulated over 6 chunks
                O = pout.tile([128, 2, D + 1], F32)
                for pc in range(2):
                    for c in range(NCH):
                        nc.tensor.matmul(
                            O[:, pc, :],
                            Ex[:, c * 256 + pc * 128 : c * 256 + pc * 128 + 128],
                            v[:, c * (D + 1) : (c + 1) * (D + 1)],
                            start=(c == 0),
                            stop=(c == NCH - 1),
                        )

                r = spool_small.tile([128, 2], F32)
                nc.vector.reciprocal(r[:], O[:, :, D])
                for pc in range(2):
                    idx = (e * B + b) * 2 + pc
                    dst = acc[:, idx * 128 : (idx + 1) * 128]
                    if x == 0:
                        nc.vector.tensor_scalar(
                            dst,
                            O[:, pc, 0:D],
                            r[:, pc : pc + 1],
                            wts_tile[:, x : x + 1],
                            mybir.AluOpType.mult,
                            mybir.AluOpType.mult,
                        )
                    else:
                        tmp = spool_small.tile([128, D], F32, tag="tmp")
                        nc.vector.tensor_scalar(
                            tmp[:],
                            O[:, pc, 0:D],
                            r[:, pc : pc + 1],
                            wts_tile[:, x : x + 1],
                            mybir.AluOpType.mult,
                            mybir.AluOpType.mult,
                        )
                        nc.vector.tensor_add(dst, dst, tmp[:])

            for e in range(ELOC):
                for b in range(B):
                    for pc in range(2):
                        idx = (e * B + b) * 2 + pc
                        nc.default_dma_engine.dma_start(
                            od[e, b, pc], acc[:, idx * 128 : (idx + 1) * 128]
                        )

    nc.compile()
    return nc


_PROGRAM = None


def _program():
    global _PROGRAM
    if _PROGRAM is None:
        _PROGRAM = _build_program()
    return _PROGRAM


def _prep_core_inputs(core, Q_aff, K_aff, V, beta_fac, wts_bcast):
    """Build the per-core input arrays (pure layout + tiny scalar folding)."""
    v_host = np.empty((NT, 128, FREE_V), dtype=MM_NP)
    k_host = np.zeros((4, NCH, NBLK * 128), dtype=np.float32)
    q_host = np.zeros((4, NCH, NBLK * 1536), dtype=np.float32)

    for t, x, e, b in _tuple_iter():
        g, blk = t % 4, t // 4
        ge = ELOC * core + e
        qrow = Q_aff[x, ge, b]  # [256]
        for c in range(NCH):
            w, half = c // 2, c % 2
            er = int(ROUTES[ge, w])
            sl = slice(half * 128, (half + 1) * 128)
            v_host[t, :, c * (D + 1) : c * (D + 1) + D] = V[x, er, b, sl, :]
            v_host[t, :, c * (D + 1) + D] = 1.0
            k_host[g, c, blk * 128 : (blk + 1) * 128] = (
                K_aff[x, er, b, sl] * beta_fac[ge, w]
            )
            q_host[g, c, blk * 1536 + c * 256 : blk * 1536 + (c + 1) * 256] = qrow
    return {"v": v_host, "k": k_host, "q": q_host, "w": wts_bcast}


def kernel(Q_aff, K_aff, V, betas, temperature, fusion_weights):
    Q_aff = np.asarray(Q_aff, dtype=np.float32)
    K_aff = np.asarray(K_aff, dtype=np.float32)
    V = np.asarray(V, dtype=np.float32)
    betas = np.asarray(betas, dtype=np.float32)
    temperature = np.asarray(temperature, dtype=np.float32)
    fusion_weights = np.asarray(fusion_weights, dtype=np.float32)

    temp = abs(float(temperature[0])) + 1e-06
    # fac(e, w) = sigmoid(betas[e, route]) for cross edges, 1 for self; /temp
    sig = 1.0 / (1.0 + np.exp(-betas.astype(np.float64)))
    beta_fac = np.empty((E, W), dtype=np.float64)
    for e in range(E):
        for w in range(W):
            er = int(ROUTES[e, w])
            beta_fac[e, w] = (1.0 if er == e else sig[e, er]) / temp
    beta_fac = beta_fac.astype(np.float32)

    fw = fusion_weights.astype(np.float64)
    fw = np.exp(fw - fw.max())
    wts = (fw / fw.sum()).astype(np.float32)
    wts_bcast = np.broadcast_to(wts, (128, NDIR)).copy()

    nc = _program()
    in_maps = [
        _prep_core_inputs(c, Q_aff, K_aff, V, beta_fac, wts_bcast)
        for c in range(N_CORES)
    ]
    res = run_bass_kernel_spmd(nc, in_maps, list(range(N_CORES)))

    out = np.empty((B, E * P, D), dtype=np.float32)
    for c in range(N_CORES):
        o = res.results[c]["o"]  # [ELOC, B, 2, 128, 128]
        for e in range(ELOC):
            ge = ELOC * c + e
            out[:, ge * P : (ge + 1) * P, :] = (
                o[e].reshape(B, P, D)
            )
    return out


# revision 3
# speedup vs baseline: 2.9432x; 2.9432x over previous
"""Trainium2 Bass kernel for CantorGlobalAttention (sparse attention).

Math (per direction x, expert e, batch b):
  scores[p, k] = Q[x,e,b,p] * kappa[k]          (rank-1 outer product)
  kappa[k]     = K_aff[x, route(e,w), b, q] * fac(e,w) / temp,  k=(w,q)
  attn         = softmax_k(scores)
  out[p, :]    = attn @ V_neighbors[k, :]
  final        = sum_x softmax(fusion_weights)[x] * out_x

Device strategy (8 cores, expert-parallel, 2 experts/core, 40 (x,e,b)
tuples/core):
  - scores via PE outer products in the [k, p] (matmul-ready) orientation:
    stationary = kappa rows, moving = block-diagonalized Q rows ->
    scores [128k, 1536] in PSUM. Both operands are bf16 hi/lo split-K
    decompositions (kappa_hi*Q_hi + kappa_hi*Q_lo + kappa_lo*Q_hi), which is
    exact to ~1e-5 while running at the 1 cycle/row bf16 matmul rate.
  - one big ScalarE Exp per tuple: PSUM [128,1536] -> SBUF bf16.
  - attn @ V as 12 accumulating bf16 matmuls (lhsT = exp chunk [128,128],
    rhs = V chunk [128,129] with a ones column appended so the softmax
    denominator Z falls out of the same matmul, fp32 PSUM accumulation).
  - VectorE: reciprocal(Z), scale by wts[x]/Z, accumulate over x in SBUF.
  - host does all layout: neighbor gather, beta/temp folding into K,
    block-diag Q, hi/lo splits, fusion-weight softmax (tiny tensors only).
"""

import numpy as np
import ml_dtypes

import concourse.bass as bass
import concourse.tile as tile
from concourse import bacc, mybir
from concourse.bass_utils import run_bass_kernel_spmd

F32 = mybir.dt.float32
BF16 = mybir.dt.bfloat16
BF16_NP = ml_dtypes.bfloat16

NDIR = 5
E = 16
W = 3
D = 128
P = 256
B = 4
DEPTH = 8

N_CORES = 8
ELOC = E // N_CORES          # experts per core = 2
NT = NDIR * ELOC * B         # tuples per core = 40
NCH = W * 2                  # key chunks per tuple (w, half) = 6
KROWS = 3 * NCH              # split-K rows per tuple = 18
FREE_V = NCH * (D + 1)       # V stage free size = 774
NBLK = NT // 4               # tuple column blocks = 10


def _routes() -> np.ndarray:
    def cantor(pos: int) -> float:
        x = pos / max(1, E - 1)
        x = max(1e-06, min(x, 1.0 - 1e-06))
        val, factor = 0.0, 0.5
        for _ in range(DEPTH):
            x *= 3.0
            digit = int(x)
            x -= digit
            if digit == 2:
                val += factor
            factor *= 0.5
        return val

    coords = np.array([cantor(i) for i in range(E)], dtype=np.float32)
    routes = np.zeros((E, W), dtype=np.int32)
    for i in range(E):
        d = np.abs(coords - coords[i])
        routes[i] = np.sort(np.argsort(d, kind="stable")[:W])
    return routes


ROUTES = _routes()


def _tuple_iter():
    """(t, x, e_local, b) in x-major order (x outermost for fusion accum)."""
    t = 0
    for x in range(NDIR):
        for e in range(ELOC):
            for b in range(B):
                yield t, x, e, b
                t += 1


def _build_program():
    nc = bacc.Bacc(None)

    vd = nc.dram_tensor("v", [NT, 128, FREE_V], BF16, kind="ExternalInput")
    kd = nc.dram_tensor("k", [4, KROWS, NBLK * 128], BF16, kind="ExternalInput")
    qd = nc.dram_tensor("q", [4, KROWS, NBLK * 1536], BF16, kind="ExternalInput")
    wd = nc.dram_tensor("w", [128, NDIR], F32, kind="ExternalInput")
    od = nc.dram_tensor("o", [ELOC, B, 2, 128, 128], F32, kind="ExternalOutput")

    with tile.TileContext(nc) as tc:
        with (
            tc.tile_pool(name="const", bufs=1) as const,
            tc.tile_pool(name="vstream", bufs=4) as vpool,
            tc.tile_pool(name="exp", bufs=2) as epool,
            tc.tile_pool(name="small", bufs=4) as spool_small,
            tc.tile_pool(name="psum_s", bufs=2, space="PSUM") as pscore,
            tc.tile_pool(name="psum_o", bufs=2, space="PSUM") as pout,
        ):
            k_tile = const.tile([128, NBLK * 128], BF16)
            q_tile = const.tile([128, NBLK * 1536], BF16)
            wts_tile = const.tile([128, NDIR], F32)
            acc = const.tile([128, ELOC * B * 2 * 128], F32)

            nc.default_dma_engine.dma_start(wts_tile[:], wd[:])
            for g in range(4):
                nc.default_dma_engine.dma_start(
                    k_tile[32 * g : 32 * g + KROWS, :], kd[g]
                )
                nc.default_dma_engine.dma_start(
                    q_tile[32 * g : 32 * g + KROWS, :], qd[g]
                )

            for t, x, e, b in _tuple_iter():
                g, blk = t % 4, t // 4
                bp = 32 * g

                v = vpool.tile([128, FREE_V], BF16)
                nc.default_dma_engine.dma_start(v[:], vd[t])

                # scores [128k, 1536] = kappa_splitK.T @ Qdiag_splitK
                S = pscore.tile([128, 1536], F32)
                for m in range(3):
                    nc.tensor.matmul(
                        S[:, m * 512 : (m + 1) * 512],
                        k_tile[bp : bp + KROWS, blk * 128 : (blk + 1) * 128],
                        q_tile[
                            bp : bp + KROWS,
                            blk * 1536 + m * 512 : blk * 1536 + (m + 1) * 512,
                        ],
                        start=True,
                        stop=True,
                        tile_position=(bp, 0),
                    )

                Ex = epool.tile([128, 1536], BF16)
                nc.scalar.activation(Ex[:], S[:], mybir.ActivationFunctionType.Exp)

                # attended [p, d] (+ Z in col 128) accumulated over 6 chunks
                O = pout.tile([128, 2, D + 1], F32)
                for pc in range(2):
                    for c in range(NCH):
                        nc.tensor.matmul(
                            O[:, pc, :],
                            Ex[:, c * 256 + pc * 128 : c * 256 + pc * 128 + 128],
                            v[:, c * (D + 1) : (c + 1) * (D + 1)],
                            start=(c == 0),
                            stop=(c == NCH - 1),
                        )

                r = spool_small.tile([128, 2], F32)
                nc.vector.reciprocal(r[:], O[:, :, D])
                for pc in range(2):
                    idx = (e * B + b) * 2 + pc
                    dst = acc[:, idx * 128 : (idx + 1) * 128]
                    if x == 0:
                        nc.vector.tensor_scalar(
                            dst,
                            O[:, pc, 0:D],
                            r[:, pc : pc + 1],
                            wts_tile[:, x : x + 1],
                            mybir.AluOpType.mult,
                            mybir.AluOpType.mult,
                        )
                    else:
                        tmp = spool_small.tile([128, D], F32, tag="tmp")
                        nc.vector.tensor_scalar(
                            tmp[:],
                            O[:, pc, 0:D],
                            r[:, pc : pc + 1],
                            wts_tile[:, x : x + 1],
                            mybir.AluOpType.mult,
                            mybir.AluOpType.mult,
                        )
                        nc.vector.tensor_add(dst, dst, tmp[:])

            for e in range(ELOC):
                for b in range(B):
                    for pc in range(2):
                        idx = (e * B + b) * 2 + pc
                        nc.default_dma_engine.dma_start(
                            od[e, b, pc], acc[:, idx * 128 : (idx + 1) * 128]
                        )

    nc.compile()
    return nc


_PROGRAM = None


def _program():
    global _PROGRAM
    if _PROGRAM is None:
        _PROGRAM = _build_program()
    return _PROGRAM


def _hi_lo(a):
    """bf16 hi/lo split: a ~= hi + lo with hi, lo bf16."""
    hi = a.astype(BF16_NP)
    lo = (a - hi.astype(np.float32)).astype(BF16_NP)
    return hi, lo


def _prep_core_inputs(core, Q_aff, K_aff, V, beta_fac, wts_bcast):
    """Build the per-core input arrays (pure layout + tiny scalar folding)."""
    v_host = np.empty((NT, 128, FREE_V), dtype=BF16_NP)
    k_host = np.zeros((4, KROWS, NBLK * 128), dtype=BF16_NP)
    q_host = np.zeros((4, KROWS, NBLK * 1536), dtype=BF16_NP)

    for t, x, e, b in _tuple_iter():
        g, blk = t % 4, t // 4
        ge = ELOC * core + e
        q_hi, q_lo = _hi_lo(Q_aff[x, ge, b])  # [256] each
        for c in range(NCH):
            w, half = c // 2, c % 2
            er = int(ROUTES[ge, w])
            sl = slice(half * 128, (half + 1) * 128)
            v_host[t, :, c * (D + 1) : c * (D + 1) + D] = V[x, er, b, sl, :]
            v_host[t, :, c * (D + 1) + D] = 1.0
            kappa = K_aff[x, er, b, sl] * beta_fac[ge, w]
            k_hi, k_lo = _hi_lo(kappa)
            ks = slice(blk * 128, (blk + 1) * 128)
            k_host[g, 3 * c + 0, ks] = k_hi
            k_host[g, 3 * c + 1, ks] = k_hi
            k_host[g, 3 * c + 2, ks] = k_lo
            qs = slice(blk * 1536 + c * 256, blk * 1536 + (c + 1) * 256)
            q_host[g, 3 * c + 0, qs] = q_hi
            q_host[g, 3 * c + 1, qs] = q_lo
            q_host[g, 3 * c + 2, qs] = q_hi
    return {"v": v_host, "k": k_host, "q": q_host, "w": wts_bcast}


def kernel(Q_aff, K_aff, V, betas, temperature, fusion_weights):
    Q_aff = np.asarray(Q_aff, dtype=np.float32)
    K_aff = np.asarray(K_aff, dtype=np.float32)
    V = np.asarray(V, dtype=np.float32)
    betas = np.asarray(betas, dtype=np.float32)
    temperature = np.asarray(temperature, dtype=np.float32)
    fusion_weights = np.asarray(fusion_weights, dtype=np.float32)

    temp = abs(float(temperature[0])) + 1e-06
    # fac(e, w) = sigmoid(betas[e, route]) for cross edges, 1 for self; /temp
    sig = 1.0 / (1.0 + np.exp(-betas.astype(np.float64)))
    beta_fac = np.empty((E, W), dtype=np.float64)
    for e in range(E):
        for w in range(W):
            er = int(ROUTES[e, w])
            beta_fac[e, w] = (1.0 if er == e else sig[e, er]) / temp
    beta_fac = beta_fac.astype(np.float32)

    fw = fusion_weights.astype(np.float64)
    fw = np.exp(fw - fw.max())
    wts = (fw / fw.sum()).astype(np.float32)
    wts_bcast = np.broadcast_to(wts, (128, NDIR)).copy()

    nc = _program()
    in_maps = [
        _prep_core_inputs(c, Q_aff, K_aff, V, beta_fac, wts_bcast)
        for c in range(N_CORES)
    ]
    res = run_bass_kernel_spmd(nc, in_maps, list(range(N_CORES)))

    out = np.empty((B, E * P, D), dtype=np.float32)
    for c in range(N_CORES):
        o = res.results[c]["o"]  # [ELOC, B, 2, 128, 128]
        for e in range(ELOC):
            ge = ELOC * c + e
            out[:, ge * P : (ge + 1) * P, :] = o[e].reshape(B, P, D)
    return out
